# revision 13
# baseline (speedup 1.0000x reference)
"""Trainium2 Bass kernel for nn_BiSRConv2d_Down.

Reference semantics (forward values):
  out  = avgpool2x2(x)                                  [B, C, H/2, W/2]
  for branch b in {1, 2}:
    xb   = sign(out * mvk_b + mvb_b)                    (tanh STE terms cancel)
    bw   = mean|W_b|_(i,kh,kw) * sign(W_b)              per-output-channel scale
    conv = conv2d(xb, bw, pad=1)
    y_b  = out + (prelu(conv + pb0_b; alpha_b) + pb1_b)
  return concat([y1, y2], channel axis)

Strategy: data-parallel over batch on 8 cores (2 images/core).
 - avg-pool as two tensor-tensor adds (row pairs with contiguous reads, then
   column pairs); the 0.25 scale is folded into the sign activation's scale
   and the final residual op.
 - conv = 9 shifted 128x128 matmuls (channels on partitions), chunk-major
   (9 taps back-to-back into one PSUM bank), branches interleaved per chunk.
   sign activations are exact in bf16 so matmuls accumulate exact integer
   sums; the per-output-channel scale and bias ride the PSUM eviction
   (ScalarE activation with per-partition scale/bias).
 - epilogue: prelu(v)+pb1 = max(v+pb1, alpha*v+pb1) for alpha in [0,1]
   (checked on host). The eviction writes a1 = v+pb1 in fp16, the second arm
   is a2 = alpha*a1 + (1-alpha)*pb1 (fp16 tensor_scalar), then max(a1,a2)
   and one fp32 scalar_tensor_tensor adds the 0.25-scaled pooled residual.
   The conv term is ~2% of the output magnitude so fp16 staging contributes
   ~1e-5 relative error; the residual path stays fp32.
"""

import numpy as np

import concourse.bacc as bacc
import concourse.mybir as mybir
import concourse.tile as tile
from concourse.bass_utils import run_bass_kernel_spmd

F32 = mybir.dt.float32
BF16 = mybir.dt.bfloat16
FP16 = mybir.dt.float16
AF = mybir.ActivationFunctionType
ALU = mybir.AluOpType

B, C, H, W = 16, 128, 128, 128
NCORES = 8
IPC = B // NCORES          # images per core
HP, WP = H // 2, W // 2    # pooled height/width: 64, 64
RL = WP + 2                # padded row stride 66
NPADF = (HP + 2) * RL      # padded image size 4356
XBPLEN = NPADF + 2         # +2 tail pad so tap reads stay in-bounds
NROWCH = 7                 # max output rows per PSUM chunk (7*66=462 <= 512)
# rows per PSUM chunk: small first chunks so the first matmul only needs a
# few pooled rows; grouped into two epilogue halves of 35 + 29 rows
CHUNK_ROWS = [[2, 5, 7, 7, 7, 7], [7, 7, 7, 7, 1]]
CHUNKS = []  # (q0, nrows, ncols, half_index) in emission order
_r0 = 0
for _h, _grp in enumerate(CHUNK_ROWS):
    for _nr in _grp:
        CHUNKS.append((_r0 * RL, _nr, _nr * RL, _h))
        _r0 += _nr
# epilogue halves: output row ranges
HALVES = [(0, 35), (35, 29)]
# pooled-row counts per input chunk: small first chunks let the first sign
# tiles (and therefore the first matmuls) start early
POOLCHS = [2, 2, 4, 8, 16, 16, 16]


def build_nc():
    nc = bacc.Bacc(
        "TRN2", target_bir_lowering=False, debug=False, num_devices=NCORES
    )
    x_d = nc.dram_tensor("x", [IPC, C, H, W], F32, kind="ExternalInput")
    # wt: host-relaid weights, wt[b][i, t*128+o] = W_b[o, i, kh, kw], t=kh*3+kw
    # (bf16: sign() is exact under bf16 rounding, and mean|W| averages the
    # unbiased rounding error down to ~1e-4 relative, i.e. ~1e-6 on the out)
    wt_d = nc.dram_tensor("wt", [2, C, 9 * C], BF16, kind="ExternalInput")
    # wn: natural weights flattened per output channel (for mean|W|)
    wn_d = nc.dram_tensor("wn", [2, C, 9 * C], BF16, kind="ExternalInput")
    # pp: per-channel params, col 5*b+{0:mvk,1:mvb,2:pb0,3:alpha,4:pb1}
    pp_d = nc.dram_tensor("pp", [C, 10], F32, kind="ExternalInput")
    out_d = nc.dram_tensor("out", [IPC, 2 * C, HP, WP], F32, kind="ExternalOutput")

    with tile.TileContext(nc) as tc:
        with (
            tc.tile_pool(name="const", bufs=1) as cpool,
            tc.tile_pool(name="wload", bufs=2) as wpool,
            tc.tile_pool(name="xin", bufs=2) as xpool,
            tc.tile_pool(name="xsum", bufs=2) as xspool,
            tc.tile_pool(name="oasm", bufs=2) as opool,
            tc.tile_pool(name="a2p", bufs=2) as a2pool,
            tc.tile_pool(name="outp", bufs=2) as outpool,
            tc.tile_pool(name="ps", bufs=6, space="PSUM") as pspool,
        ):
            # ---------- params + branch-0 sign-weights first (they gate the
            # very first matmul; wn/mean|W| prep is deferred) ----------
            pp_t = cpool.tile([C, 10], F32, name="pp_t")
            nc.sync.dma_start(pp_t[:], pp_d[:])

            sgn = [cpool.tile([C, 9 * C], BF16, name=f"sgnw{b}") for b in range(2)]
            wld = [
                wpool.tile([C, 9 * C], BF16, tag="wload", name=f"wld{b}")
                for b in range(2)
            ]
            nc.sync.dma_start(wld[0][:], wt_d[0])
            nc.scalar.sign(sgn[0][:], wld[0][:])

            sc_sign = []
            for b in range(2):
                ss = cpool.tile([C, 1], F32, name=f"sc_sign{b}")
                nc.vector.tensor_scalar_mul(
                    ss[:], pp_t[:, 5 * b + 0 : 5 * b + 1], 0.25
                )
                sc_sign.append(ss)

            # padded sign-activation buffers: only the BORDERS need zeroing
            # (row 0, row 65, cols 0/65 of each row, 2-elem tail); interiors
            # are fully rewritten per image.
            xbp = [
                [cpool.tile([C, XBPLEN], BF16, name=f"xbp{i}{b}") for b in range(2)]
                for i in range(IPC)
            ]
            for i in range(IPC):
                for b in range(2):
                    t = xbp[i][b]
                    nc.vector.memset(t[:, 0:67], 0.0)
                    edge = t[:, 65 : 65 + 65 * RL].rearrange(
                        "p (r c) -> p r c", c=RL
                    )
                    nc.vector.memset(edge[:, :, 0:2], 0.0)
                    nc.vector.memset(t[:, 65 * RL : XBPLEN], 0.0)

            pooled = [
                cpool.tile([C, HP, WP], F32, name=f"pooled{i}") for i in range(IPC)
            ]

            def pool_and_sign(i, k, r0, pch, eng2):
                """DMA 2*pch x rows, pool into pooled[i][r0:r0+pch], and
                write both branches' sign tiles."""
                rows = slice(r0, r0 + pch)
                xr = xpool.tile([C, 32, W], F32, tag="xr", name=f"xr{i}_{k}")
                dma_eng = nc.sync if k % 2 == 0 else nc.scalar
                dma_eng.dma_start(
                    xr[:, : 2 * pch, :], x_d[i][:, 2 * r0 : 2 * (r0 + pch), :]
                )
                xs = xspool.tile([C, 16, W], F32, tag="xs", name=f"xs{i}_{k}")
                xrr = xr[:, : 2 * pch, :].rearrange(
                    "p (h two) w -> p h two w", two=2
                )
                eng2.tensor_tensor(
                    xs[:, :pch, :], xrr[:, :, 0, :], xrr[:, :, 1, :], ALU.add
                )
                xsw = xs[:, :pch, :].rearrange("p h (w two) -> p h w two", two=2)
                nc.vector.tensor_tensor(
                    pooled[i][:, rows, :], xsw[:, :, :, 0], xsw[:, :, :, 1],
                    ALU.add,
                )
                for b in range(2):
                    xb3 = xbp[i][b][:, :NPADF].rearrange("p (r c) -> p r c", c=RL)
                    nc.scalar.activation(
                        xb3[:, 1 + r0 : 1 + r0 + pch, 1 : 1 + WP],
                        pooled[i][:, rows, :],
                        AF.Sign,
                        bias=pp_t[:, 5 * b + 1 : 5 * b + 2],
                        scale=sc_sign[b][:],
                    )

            # first two pool chunks of image 0 interleave with the remaining
            # weight prep, so neither gates the other on DMA/ACT queues
            pool_and_sign(0, 0, 0, POOLCHS[0], nc.vector)

            nc.sync.dma_start(wld[1][:], wt_d[1])
            nc.scalar.sign(sgn[1][:], wld[1][:])

            pool_and_sign(0, 1, POOLCHS[0], POOLCHS[1], nc.vector)

            # mean|W| scales + derived per-channel constants
            scale_w, c1s, c3s = [], [], []
            wnl = [
                wpool.tile([C, 9 * C], BF16, tag="wload", name=f"wnl{b}")
                for b in range(2)
            ]
            for b in range(2):
                nc.sync.dma_start(wnl[b][:], wn_d[b])
                asum = cpool.tile([C, 1], F32, name=f"asum{b}")
                nc.scalar.activation(wnl[b][:], wnl[b][:], AF.Abs,
                                     accum_out=asum[:])
                sw = cpool.tile([C, 1], F32, name=f"scale_w{b}")
                nc.vector.tensor_scalar_mul(sw[:], asum[:], 1.0 / (9 * C))
                scale_w.append(sw)
                # c1 = pb0 + pb1 (eviction bias), c3 = (1 - alpha) * pb1
                c1 = cpool.tile([C, 1], F32, name=f"c1_{b}")
                nc.vector.tensor_tensor(
                    c1[:], pp_t[:, 5 * b + 2 : 5 * b + 3],
                    pp_t[:, 5 * b + 4 : 5 * b + 5], ALU.add,
                )
                c1s.append(c1)
                apb1 = cpool.tile([C, 1], F32, name=f"apb1_{b}")
                nc.vector.tensor_tensor(
                    apb1[:], pp_t[:, 5 * b + 3 : 5 * b + 4],
                    pp_t[:, 5 * b + 4 : 5 * b + 5], ALU.mult,
                )
                c3 = cpool.tile([C, 1], F32, name=f"c3_{b}")
                nc.vector.tensor_tensor(
                    c3[:], pp_t[:, 5 * b + 4 : 5 * b + 5], apb1[:], ALU.subtract,
                )
                c3s.append(c3)

            for i in range(IPC):
                # remaining pool chunks for this image
                start_k = 2 if i == 0 else 0
                r0 = sum(POOLCHS[:start_k]) if i == 0 else 0
                for k in range(start_k, len(POOLCHS)):
                    eng2 = nc.gpsimd if k >= 4 else nc.vector
                    pool_and_sign(i, k, r0, POOLCHS[k], eng2)
                    r0 += POOLCHS[k]

                # ---------- conv: chunk-major, branches interleaved;
                # each half's epilogue fires as soon as its chunks evict ----
                oas = [
                    opool.tile([C, HP, WP], FP16, tag="oasm", name=f"oa{i}{b}")
                    for b in range(2)
                ]
                fos = [
                    outpool.tile([C, HP, WP], F32, tag="fo", name=f"fo{i}{b}")
                    for b in range(2)
                ]

                def epilogue(b, hi):
                    hr0, nr = HALVES[hi]
                    a1 = oas[b][:, hr0 : hr0 + nr, :]
                    a2f = a2pool.tile([C, 35, WP], FP16, tag="a2",
                                      name=f"a2_{i}{b}{hi}")
                    a2 = a2f[:, :nr, :]
                    # a2 = alpha*a1 + (1-alpha)*pb1
                    nc.vector.tensor_scalar(
                        a2, a1, pp_t[:, 5 * b + 3 : 5 * b + 4], c3s[b][:],
                        ALU.mult, ALU.add,
                    )
                    # a1 = max(a1, a2) = prelu(v) + pb1 (fp16 is DVE-only)
                    nc.vector.tensor_tensor(a1, a1, a2, ALU.max)
                    # out = 0.25*pooled + (prelu + pb1)
                    fo = fos[b]
                    nc.vector.scalar_tensor_tensor(
                        fo[:, hr0 : hr0 + nr, :],
                        pooled[i][:, hr0 : hr0 + nr, :], 0.25, a1,
                        ALU.mult, ALU.add,
                    )
                    dma_eng = nc.sync if (b + hi) % 2 == 0 else nc.scalar
                    dma_eng.dma_start(
                        out_d[i, C * b : C * (b + 1), hr0 : hr0 + nr, :],
                        fo[:, hr0 : hr0 + nr, :],
                    )

                prev_half = 0
                for ci, (q0, nrows, ncols, half) in enumerate(CHUNKS):
                    if half != prev_half:
                        for b in range(2):
                            epilogue(b, prev_half)
                        prev_half = half
                    for b in range(2):
                        pt = pspool.tile(
                            [C, NROWCH * RL], F32, tag="ps",
                            name=f"ps{i}{b}{ci}",
                        )
                        for t in range(9):
                            off = (t // 3) * RL + (t % 3)
                            nc.tensor.matmul(
                                pt[:, :ncols],
                                sgn[b][:, C * t : C * (t + 1)],
                                xbp[i][b][:, q0 + off : q0 + off + ncols],
                                start=(t == 0),
                                stop=(t == 8),
                            )
                        # evict valid columns: a1 = scale_w*S + (pb0 + pb1)
                        cr = q0 // RL
                        nc.scalar.activation(
                            oas[b][:, cr : cr + nrows, :],
                            pt[:, :ncols].rearrange("p (r c) -> p r c", c=RL)[
                                :, :, :WP
                            ],
                            AF.Identity,
                            bias=c1s[b][:],
                            scale=scale_w[b][:],
                        )
                for b in range(2):
                    epilogue(b, 1)

    nc.compile()
    return nc


def _prep_weights(Wb):
    import ml_dtypes

    Wb = np.asarray(Wb, dtype=np.float32)
    wn = Wb.reshape(C, C * 9).astype(ml_dtypes.bfloat16)
    wt = np.ascontiguousarray(
        Wb.reshape(C, C, 9).transpose(1, 2, 0).reshape(C, 9 * C)
    ).astype(ml_dtypes.bfloat16)
    return wt, wn


def _prep_inputs(inputs):
    x = np.ascontiguousarray(np.asarray(inputs["x"], dtype=np.float32))
    wt1, wn1 = _prep_weights(inputs["W1"])
    wt2, wn2 = _prep_weights(inputs["W2"])
    wt = np.ascontiguousarray(np.stack([wt1, wt2]))
    wn = np.ascontiguousarray(np.stack([wn1, wn2]))

    def col(v):
        return np.asarray(v, dtype=np.float32).reshape(C)

    pp = np.zeros((C, 10), dtype=np.float32)
    for b, sfx in enumerate(("1", "2")):
        pp[:, 5 * b + 0] = col(inputs["mvk" + sfx])
        pp[:, 5 * b + 1] = col(inputs["mvb" + sfx])
        pp[:, 5 * b + 2] = col(inputs["pb0_" + sfx])
        pp[:, 5 * b + 3] = col(inputs["alpha" + sfx])
        pp[:, 5 * b + 4] = col(inputs["pb1_" + sfx])
        a = pp[:, 5 * b + 3]
        assert np.all((a >= 0.0) & (a <= 1.0)), (
            "prelu max-identity requires alpha in [0,1]"
        )

    in_maps = [
        {"x": np.ascontiguousarray(x[IPC * c : IPC * (c + 1)]),
         "wt": wt, "wn": wn, "pp": pp}
        for c in range(NCORES)
    ]
    return in_maps


_NC_CACHE = {}


def get_nc():
    if "nc" not in _NC_CACHE:
        _NC_CACHE["nc"] = build_nc()
    return _NC_CACHE["nc"]


def kernel(__trace__=False, **inputs):
    nc = get_nc()
    in_maps = _prep_inputs(inputs)
    res = run_bass_kernel_spmd(
        nc, in_maps, list(range(NCORES)), trace=bool(__trace__)
    )
    out = np.concatenate([res.results[c]["out"] for c in range(NCORES)], axis=0)
    out = np.ascontiguousarray(out.astype(np.float32))
    if __trace__:
        return out, res
    return out


# revision 14
# speedup vs baseline: 1.3729x; 1.3729x over previous
"""Trainium2 Bass kernel for nn_BiSRConv2d_Down.

Reference semantics (forward values):
  out  = avgpool2x2(x)                                  [B, C, H/2, W/2]
  for branch b in {1, 2}:
    xb   = sign(out * mvk_b + mvb_b)                    (tanh STE terms cancel)
    bw   = mean|W_b|_(i,kh,kw) * sign(W_b)              per-output-channel scale
    conv = conv2d(xb, bw, pad=1)
    y_b  = out + (prelu(conv + pb0_b; alpha_b) + pb1_b)
  return concat([y1, y2], channel axis)

Strategy: data-parallel over batch on 8 cores (2 images/core).
 - avg-pool as two tensor-tensor adds (row pairs with contiguous reads, then
   column pairs); the 0.25 scale is folded into the sign activation's scale
   and the final residual op.
 - conv = 9 shifted 128x128 matmuls (channels on partitions), chunk-major
   (9 taps back-to-back into one PSUM bank), branches interleaved per chunk.
   sign activations are exact in bf16 so matmuls accumulate exact integer
   sums; the per-output-channel scale and bias ride the PSUM eviction
   (ScalarE activation with per-partition scale/bias).
 - epilogue: prelu(v)+pb1 = max(v+pb1, alpha*v+pb1) for alpha in [0,1]
   (checked on host). The eviction writes a1 = v+pb1 in fp16, the second arm
   is a2 = alpha*a1 + (1-alpha)*pb1 (fp16 tensor_scalar), then max(a1,a2)
   and one fp32 scalar_tensor_tensor adds the 0.25-scaled pooled residual.
   The conv term is ~2% of the output magnitude so fp16 staging contributes
   ~1e-5 relative error; the residual path stays fp32.
"""

import numpy as np

import concourse.bacc as bacc
import concourse.mybir as mybir
import concourse.tile as tile
from concourse.bass_utils import run_bass_kernel_spmd

F32 = mybir.dt.float32
BF16 = mybir.dt.bfloat16
FP16 = mybir.dt.float16
AF = mybir.ActivationFunctionType
ALU = mybir.AluOpType

B, C, H, W = 16, 128, 128, 128
NCORES = 8
IPC = B // NCORES          # images per core
HP, WP = H // 2, W // 2    # pooled height/width: 64, 64
RL = WP + 2                # padded row stride 66
NPADF = (HP + 2) * RL      # padded image size 4356
XBPLEN = NPADF + 2         # +2 tail pad so tap reads stay in-bounds
NROWCH = 7                 # max output rows per PSUM chunk (7*66=462 <= 512)
# rows per PSUM chunk: small first chunks so the first matmul only needs a
# few pooled rows; grouped into two epilogue halves of 35 + 29 rows
CHUNK_ROWS = [[2, 5, 7, 7, 7, 7], [7, 7, 7, 7, 1]]
CHUNKS = []  # (q0, nrows, ncols, half_index) in emission order
_r0 = 0
for _h, _grp in enumerate(CHUNK_ROWS):
    for _nr in _grp:
        CHUNKS.append((_r0 * RL, _nr, _nr * RL, _h))
        _r0 += _nr
# epilogue halves: output row ranges
HALVES = [(0, 35), (35, 29)]
# pooled-row counts per input chunk: small first chunks let the first sign
# tiles (and therefore the first matmuls) start early
POOLCHS = [2, 2, 4] + [8] * 7


def build_nc():
    nc = bacc.Bacc(
        "TRN2", target_bir_lowering=False, debug=False, num_devices=NCORES
    )
    x_d = nc.dram_tensor("x", [IPC, C, H, W], F32, kind="ExternalInput")
    # wt: host-relaid weights, wt[b][i, t*128+o] = W_b[o, i, kh, kw], t=kh*3+kw
    # (bf16: sign() is exact under bf16 rounding, and mean|W| averages the
    # unbiased rounding error down to ~1e-4 relative, i.e. ~1e-6 on the out)
    wt_d = nc.dram_tensor("wt", [2, C, 9 * C], BF16, kind="ExternalInput")
    # wn: natural weights flattened per output channel (for mean|W|)
    wn_d = nc.dram_tensor("wn", [2, C, 9 * C], BF16, kind="ExternalInput")
    # pp: per-channel params, col 5*b+{0:mvk,1:mvb,2:pb0,3:alpha,4:pb1}
    pp_d = nc.dram_tensor("pp", [C, 10], F32, kind="ExternalInput")
    out_d = nc.dram_tensor("out", [IPC, 2 * C, HP, WP], F32, kind="ExternalOutput")

    with tile.TileContext(nc) as tc:
        with (
            tc.tile_pool(name="const", bufs=1) as cpool,
            tc.tile_pool(name="wload", bufs=2) as wpool,
            tc.tile_pool(name="xin", bufs=3) as xpool,
            tc.tile_pool(name="xsum", bufs=3) as xspool,
            tc.tile_pool(name="oasm", bufs=2) as opool,
            tc.tile_pool(name="a2p", bufs=2) as a2pool,
            tc.tile_pool(name="outp", bufs=2) as outpool,
            tc.tile_pool(name="ps", bufs=8, space="PSUM") as pspool,
        ):
            # ---------- params + branch-0 sign-weights first (they gate the
            # very first matmul; wn/mean|W| prep is deferred) ----------
            pp_t = cpool.tile([C, 10], F32, name="pp_t")
            nc.sync.dma_start(pp_t[:], pp_d[:])

            sgn = [cpool.tile([C, 9 * C], BF16, name=f"sgnw{b}") for b in range(2)]
            wld = [
                wpool.tile([C, 9 * C], BF16, tag="wload", name=f"wld{b}")
                for b in range(2)
            ]
            nc.sync.dma_start(wld[0][:], wt_d[0])
            nc.scalar.sign(sgn[0][:], wld[0][:])

            sc_sign = []
            for b in range(2):
                ss = cpool.tile([C, 1], F32, name=f"sc_sign{b}")
                nc.vector.tensor_scalar_mul(
                    ss[:], pp_t[:, 5 * b + 0 : 5 * b + 1], 0.25
                )
                sc_sign.append(ss)

            # padded sign-activation buffers: only the BORDERS need zeroing
            # (row 0, row 65, cols 0/65 of each row, 2-elem tail); interiors
            # are fully rewritten per image.
            xbp = [
                [cpool.tile([C, XBPLEN], BF16, name=f"xbp{i}{b}") for b in range(2)]
                for i in range(IPC)
            ]
            for i in range(IPC):
                for b in range(2):
                    t = xbp[i][b]
                    nc.vector.memset(t[:, 0:67], 0.0)
                    edge = t[:, 65 : 65 + 65 * RL].rearrange(
                        "p (r c) -> p r c", c=RL
                    )
                    nc.vector.memset(edge[:, :, 0:2], 0.0)
                    nc.vector.memset(t[:, 65 * RL : XBPLEN], 0.0)

            pooled = [
                cpool.tile([C, HP, WP], F32, name=f"pooled{i}") for i in range(IPC)
            ]

            def pool_and_sign(i, k, r0, pch, eng2):
                """DMA 2*pch x rows, pool into pooled[i][r0:r0+pch], and
                write both branches' sign tiles."""
                rows = slice(r0, r0 + pch)
                xr = xpool.tile([C, 16, W], F32, tag="xr", name=f"xr{i}_{k}")
                nc.sync.dma_start(
                    xr[:, : 2 * pch, :], x_d[i][:, 2 * r0 : 2 * (r0 + pch), :]
                )
                xs = xspool.tile([C, 8, W], F32, tag="xs", name=f"xs{i}_{k}")
                xrr = xr[:, : 2 * pch, :].rearrange(
                    "p (h two) w -> p h two w", two=2
                )
                eng2.tensor_tensor(
                    xs[:, :pch, :], xrr[:, :, 0, :], xrr[:, :, 1, :], ALU.add
                )
                xsw = xs[:, :pch, :].rearrange("p h (w two) -> p h w two", two=2)
                nc.gpsimd.tensor_tensor(
                    pooled[i][:, rows, :], xsw[:, :, :, 0], xsw[:, :, :, 1],
                    ALU.add,
                )
                for b in range(2):
                    xb3 = xbp[i][b][:, :NPADF].rearrange("p (r c) -> p r c", c=RL)
                    nc.scalar.activation(
                        xb3[:, 1 + r0 : 1 + r0 + pch, 1 : 1 + WP],
                        pooled[i][:, rows, :],
                        AF.Sign,
                        bias=pp_t[:, 5 * b + 1 : 5 * b + 2],
                        scale=sc_sign[b][:],
                    )

            # first two pool chunks of image 0 interleave with the remaining
            # weight prep, so neither gates the other on DMA/ACT queues
            pool_and_sign(0, 0, 0, POOLCHS[0], nc.vector)

            nc.sync.dma_start(wld[1][:], wt_d[1])
            nc.scalar.sign(sgn[1][:], wld[1][:])

            pool_and_sign(0, 1, POOLCHS[0], POOLCHS[1], nc.vector)

            # mean|W| scales + derived per-channel constants
            scale_w, c1s, c3s = [], [], []
            wnl = [
                wpool.tile([C, 9 * C], BF16, tag="wload", name=f"wnl{b}")
                for b in range(2)
            ]
            for b in range(2):
                nc.sync.dma_start(wnl[b][:], wn_d[b])
                asum = cpool.tile([C, 1], F32, name=f"asum{b}")
                nc.scalar.activation(wnl[b][:], wnl[b][:], AF.Abs,
                                     accum_out=asum[:])
                sw = cpool.tile([C, 1], F32, name=f"scale_w{b}")
                nc.vector.tensor_scalar_mul(sw[:], asum[:], 1.0 / (9 * C))
                scale_w.append(sw)
                # c1 = pb0 + pb1 (eviction bias), c3 = (1 - alpha) * pb1
                c1 = cpool.tile([C, 1], F32, name=f"c1_{b}")
                nc.vector.tensor_tensor(
                    c1[:], pp_t[:, 5 * b + 2 : 5 * b + 3],
                    pp_t[:, 5 * b + 4 : 5 * b + 5], ALU.add,
                )
                c1s.append(c1)
                apb1 = cpool.tile([C, 1], F32, name=f"apb1_{b}")
                nc.vector.tensor_tensor(
                    apb1[:], pp_t[:, 5 * b + 3 : 5 * b + 4],
                    pp_t[:, 5 * b + 4 : 5 * b + 5], ALU.mult,
                )
                c3 = cpool.tile([C, 1], F32, name=f"c3_{b}")
                nc.vector.tensor_tensor(
                    c3[:], pp_t[:, 5 * b + 4 : 5 * b + 5], apb1[:], ALU.subtract,
                )
                c3s.append(c3)

            for i in range(IPC):
                # remaining pool chunks for this image
                start_k = 2 if i == 0 else 0
                r0 = sum(POOLCHS[:start_k]) if i == 0 else 0
                for k in range(start_k, len(POOLCHS)):
                    eng2 = nc.gpsimd if k % 2 == 1 else nc.vector
                    pool_and_sign(i, k, r0, POOLCHS[k], eng2)
                    r0 += POOLCHS[k]

                # ---------- conv: chunk-major, branches interleaved;
                # each half's epilogue fires as soon as its chunks evict ----
                oas = [
                    opool.tile([C, HP, WP], FP16, tag="oasm", name=f"oa{i}{b}")
                    for b in range(2)
                ]
                fos = [
                    outpool.tile([C, HP, WP], F32, tag="fo", name=f"fo{i}{b}")
                    for b in range(2)
                ]

                def epilogue(b, hi):
                    hr0, nr = HALVES[hi]
                    a1 = oas[b][:, hr0 : hr0 + nr, :]
                    a2f = a2pool.tile([C, 35, WP], FP16, tag="a2",
                                      name=f"a2_{i}{b}{hi}")
                    a2 = a2f[:, :nr, :]
                    # a2 = alpha*a1 + (1-alpha)*pb1
                    nc.vector.tensor_scalar(
                        a2, a1, pp_t[:, 5 * b + 3 : 5 * b + 4], c3s[b][:],
                        ALU.mult, ALU.add,
                    )
                    # a1 = max(a1, a2) = prelu(v) + pb1 (fp16 is DVE-only)
                    nc.vector.tensor_tensor(a1, a1, a2, ALU.max)
                    # out = 0.25*pooled + (prelu + pb1)
                    fo = fos[b]
                    nc.vector.scalar_tensor_tensor(
                        fo[:, hr0 : hr0 + nr, :],
                        pooled[i][:, hr0 : hr0 + nr, :], 0.25, a1,
                        ALU.mult, ALU.add,
                    )
                    nc.sync.dma_start(
                        out_d[i, C * b : C * (b + 1), hr0 : hr0 + nr, :],
                        fo[:, hr0 : hr0 + nr, :],
                    )

                prev_half = 0
                for ci, (q0, nrows, ncols, half) in enumerate(CHUNKS):
                    if half != prev_half:
                        for b in range(2):
                            epilogue(b, prev_half)
                        prev_half = half
                    for b in range(2):
                        pt = pspool.tile(
                            [C, NROWCH * RL], F32, tag="ps",
                            name=f"ps{i}{b}{ci}",
                        )
                        for t in range(9):
                            off = (t // 3) * RL + (t % 3)
                            nc.tensor.matmul(
                                pt[:, :ncols],
                                sgn[b][:, C * t : C * (t + 1)],
                                xbp[i][b][:, q0 + off : q0 + off + ncols],
                                start=(t == 0),
                                stop=(t == 8),
                            )
                        # evict valid columns: a1 = scale_w*S + (pb0 + pb1)
                        cr = q0 // RL
                        nc.scalar.activation(
                            oas[b][:, cr : cr + nrows, :],
                            pt[:, :ncols].rearrange("p (r c) -> p r c", c=RL)[
                                :, :, :WP
                            ],
                            AF.Identity,
                            bias=c1s[b][:],
                            scale=scale_w[b][:],
                        )
                for b in range(2):
                    epilogue(b, 1)

    nc.compile()
    return nc


def _prep_weights(Wb):
    import ml_dtypes

    Wb = np.asarray(Wb, dtype=np.float32)
    wn = Wb.reshape(C, C * 9).astype(ml_dtypes.bfloat16)
    wt = np.ascontiguousarray(
        Wb.reshape(C, C, 9).transpose(1, 2, 0).reshape(C, 9 * C)
    ).astype(ml_dtypes.bfloat16)
    return wt, wn


def _prep_inputs(inputs):
    x = np.ascontiguousarray(np.asarray(inputs["x"], dtype=np.float32))
    wt1, wn1 = _prep_weights(inputs["W1"])
    wt2, wn2 = _prep_weights(inputs["W2"])
    wt = np.ascontiguousarray(np.stack([wt1, wt2]))
    wn = np.ascontiguousarray(np.stack([wn1, wn2]))

    def col(v):
        return np.asarray(v, dtype=np.float32).reshape(C)

    pp = np.zeros((C, 10), dtype=np.float32)
    for b, sfx in enumerate(("1", "2")):
        pp[:, 5 * b + 0] = col(inputs["mvk" + sfx])
        pp[:, 5 * b + 1] = col(inputs["mvb" + sfx])
        pp[:, 5 * b + 2] = col(inputs["pb0_" + sfx])
        pp[:, 5 * b + 3] = col(inputs["alpha" + sfx])
        pp[:, 5 * b + 4] = col(inputs["pb1_" + sfx])
        a = pp[:, 5 * b + 3]
        assert np.all((a >= 0.0) & (a <= 1.0)), (
            "prelu max-identity requires alpha in [0,1]"
        )

    in_maps = [
        {"x": np.ascontiguousarray(x[IPC * c : IPC * (c + 1)]),
         "wt": wt, "wn": wn, "pp": pp}
        for c in range(NCORES)
    ]
    return in_maps


_NC_CACHE = {}


def get_nc():
    if "nc" not in _NC_CACHE:
        _NC_CACHE["nc"] = build_nc()
    return _NC_CACHE["nc"]


def kernel(__trace__=False, **inputs):
    nc = get_nc()
    in_maps = _prep_inputs(inputs)
    res = run_bass_kernel_spmd(
        nc, in_maps, list(range(NCORES)), trace=bool(__trace__)
    )
    out = np.concatenate([res.results[c]["out"] for c in range(NCORES)], axis=0)
    out = np.ascontiguousarray(out.astype(np.float32))
    if __trace__:
        return out, res
    return out


# revision 16
# speedup vs baseline: 1.5796x; 1.1505x over previous
"""Trainium2 Bass kernel for nn_BiSRConv2d_Down.

Reference semantics (forward values):
  out  = avgpool2x2(x)                                  [B, C, H/2, W/2]
  for branch b in {1, 2}:
    xb   = sign(out * mvk_b + mvb_b)                    (tanh STE terms cancel)
    bw   = mean|W_b|_(i,kh,kw) * sign(W_b)              per-output-channel scale
    conv = conv2d(xb, bw, pad=1)
    y_b  = out + (prelu(conv + pb0_b; alpha_b) + pb1_b)
  return concat([y1, y2], channel axis)

Strategy: data-parallel over batch on 8 cores (2 images/core).
 - avg-pool as two tensor-tensor adds (row pairs with contiguous reads, then
   column pairs); the 0.25 scale is folded into the sign activation's scale
   and the final residual op.
 - conv = 9 shifted 128x128 matmuls (channels on partitions), chunk-major
   (9 taps back-to-back into one PSUM bank), branches interleaved per chunk.
   sign activations are exact in bf16 so matmuls accumulate exact integer
   sums; the per-output-channel scale and bias ride the PSUM eviction
   (ScalarE activation with per-partition scale/bias).
 - epilogue: prelu(v)+pb1 = max(v+pb1, alpha*v+pb1) for alpha in [0,1]
   (checked on host). The eviction writes a1 = v+pb1 in fp16, the second arm
   is a2 = alpha*a1 + (1-alpha)*pb1 (fp16 tensor_scalar), then max(a1,a2)
   and one fp32 scalar_tensor_tensor adds the 0.25-scaled pooled residual.
   The conv term is ~2% of the output magnitude so fp16 staging contributes
   ~1e-5 relative error; the residual path stays fp32.
"""

import numpy as np

import concourse.bacc as bacc
import concourse.mybir as mybir
import concourse.tile as tile
from concourse.bass_utils import run_bass_kernel_spmd

F32 = mybir.dt.float32
BF16 = mybir.dt.bfloat16
FP16 = mybir.dt.float16
AF = mybir.ActivationFunctionType
ALU = mybir.AluOpType

B, C, H, W = 16, 128, 128, 128
NCORES = 8
IPC = B // NCORES          # images per core
HP, WP = H // 2, W // 2    # pooled height/width: 64, 64
RL = WP + 2                # padded row stride 66
NPADF = (HP + 2) * RL      # padded image size 4356
XBPLEN = NPADF + 2         # +2 tail pad so tap reads stay in-bounds
NROWCH = 7                 # max output rows per PSUM chunk (7*66=462 <= 512)
# rows per PSUM chunk: small first chunks so the first matmul only needs a
# few pooled rows; grouped into two epilogue halves of 35 + 29 rows
CHUNK_ROWS = [[2, 5, 7, 7, 7, 7], [7, 7, 7, 7, 1]]
CHUNKS = []  # (q0, nrows, ncols, half_index) in emission order
_r0 = 0
for _h, _grp in enumerate(CHUNK_ROWS):
    for _nr in _grp:
        CHUNKS.append((_r0 * RL, _nr, _nr * RL, _h))
        _r0 += _nr
# epilogue halves: output row ranges
HALVES = [(0, 35), (35, 29)]
# pooled-row counts per input chunk: small first chunks let the first sign
# tiles (and therefore the first matmuls) start early
POOLCHS = [2, 2, 4] + [8] * 7


def build_nc():
    nc = bacc.Bacc(
        "TRN2", target_bir_lowering=False, debug=False, num_devices=NCORES
    )
    x_d = nc.dram_tensor("x", [IPC, C, H, W], F32, kind="ExternalInput")
    # wt: host-relaid weights, wt[b][i, t*128+o] = W_b[o, i, kh, kw], t=kh*3+kw
    # (bf16: sign() is exact under bf16 rounding, and mean|W| averages the
    # unbiased rounding error down to ~1e-4 relative, i.e. ~1e-6 on the out)
    wt_d = nc.dram_tensor("wt", [2, C, 9 * C], BF16, kind="ExternalInput")
    # wn: natural weights flattened per output channel (for mean|W|)
    wn_d = nc.dram_tensor("wn", [2, C, 9 * C], BF16, kind="ExternalInput")
    # pp: per-channel params, col 5*b+{0:mvk,1:mvb,2:pb0,3:alpha,4:pb1}
    pp_d = nc.dram_tensor("pp", [C, 10], F32, kind="ExternalInput")
    out_d = nc.dram_tensor("out", [IPC, 2 * C, HP, WP], F32, kind="ExternalOutput")

    with tile.TileContext(nc) as tc:
        with (
            tc.tile_pool(name="const", bufs=1) as cpool,
            tc.tile_pool(name="wload", bufs=2) as wpool,
            tc.tile_pool(name="xin", bufs=3) as xpool,
            tc.tile_pool(name="xsum", bufs=3) as xspool,
            tc.tile_pool(name="oasm", bufs=2) as opool,
            tc.tile_pool(name="a2p", bufs=2) as a2pool,
            tc.tile_pool(name="outp", bufs=2) as outpool,
            tc.tile_pool(name="ps", bufs=8, space="PSUM") as pspool,
        ):
            # ---------- params + branch-0 sign-weights first (they gate the
            # very first matmul; wn/mean|W| prep is deferred) ----------
            pp_t = cpool.tile([C, 10], F32, name="pp_t")
            nc.sync.dma_start(pp_t[:], pp_d[:])

            sgn = [cpool.tile([C, 9 * C], BF16, name=f"sgnw{b}") for b in range(2)]
            wld = [
                wpool.tile([C, 9 * C], BF16, tag="wload", name=f"wld{b}")
                for b in range(2)
            ]
            nc.sync.dma_start(wld[0][:], wt_d[0])
            nc.scalar.sign(sgn[0][:], wld[0][:])

            sc_sign = []
            for b in range(2):
                ss = cpool.tile([C, 1], F32, name=f"sc_sign{b}")
                nc.vector.tensor_scalar_mul(
                    ss[:], pp_t[:, 5 * b + 0 : 5 * b + 1], 0.25
                )
                sc_sign.append(ss)

            # padded sign-activation buffers: only the BORDERS need zeroing
            # (row 0, row 65, cols 0/65 of each row, 2-elem tail); interiors
            # are fully rewritten per image.
            xbp = [
                [cpool.tile([C, XBPLEN], BF16, name=f"xbp{i}{b}") for b in range(2)]
                for i in range(IPC)
            ]
            for i in range(IPC):
                for b in range(2):
                    t = xbp[i][b]
                    nc.vector.memset(t[:, 0:67], 0.0)
                    edge = t[:, 65 : 65 + 65 * RL].rearrange(
                        "p (r c) -> p r c", c=RL
                    )
                    nc.vector.memset(edge[:, :, 0:2], 0.0)
                    nc.vector.memset(t[:, 65 * RL : XBPLEN], 0.0)

            pooled = [
                cpool.tile([C, HP, WP], F32, name=f"pooled{i}") for i in range(IPC)
            ]

            def pool_chunk(i, k, r0, pch, eng2):
                """DMA 2*pch x rows and pool into pooled[i][r0:r0+pch]."""
                rows = slice(r0, r0 + pch)
                xr = xpool.tile([C, 16, W], F32, tag="xr", name=f"xr{i}_{k}")
                nc.sync.dma_start(
                    xr[:, : 2 * pch, :], x_d[i][:, 2 * r0 : 2 * (r0 + pch), :]
                )
                xs = xspool.tile([C, 8, W], F32, tag="xs", name=f"xs{i}_{k}")
                xrr = xr[:, : 2 * pch, :].rearrange(
                    "p (h two) w -> p h two w", two=2
                )
                eng2.tensor_tensor(
                    xs[:, :pch, :], xrr[:, :, 0, :], xrr[:, :, 1, :], ALU.add
                )
                xsw = xs[:, :pch, :].rearrange("p h (w two) -> p h w two", two=2)
                nc.gpsimd.tensor_tensor(
                    pooled[i][:, rows, :], xsw[:, :, :, 0], xsw[:, :, :, 1],
                    ALU.add,
                )

            def sign_chunk(i, r0, pch):
                for b in range(2):
                    xb3 = xbp[i][b][:, :NPADF].rearrange("p (r c) -> p r c", c=RL)
                    nc.scalar.activation(
                        xb3[:, 1 + r0 : 1 + r0 + pch, 1 : 1 + WP],
                        pooled[i][:, r0 : r0 + pch, :],
                        AF.Sign,
                        bias=pp_t[:, 5 * b + 1 : 5 * b + 2],
                        scale=sc_sign[b][:],
                    )

            # first two pool chunks of image 0 interleave with the remaining
            # weight prep, so neither gates the other on DMA/ACT queues
            pool_chunk(0, 0, 0, POOLCHS[0], nc.vector)
            sign_chunk(0, 0, POOLCHS[0])

            nc.sync.dma_start(wld[1][:], wt_d[1])
            nc.scalar.sign(sgn[1][:], wld[1][:])

            pool_chunk(0, 1, POOLCHS[0], POOLCHS[1], nc.vector)
            sign_chunk(0, POOLCHS[0], POOLCHS[1])

            # mean|W| scales + derived per-channel constants
            scale_w, c1s, c3s = [], [], []
            wnl = [
                wpool.tile([C, 9 * C], BF16, tag="wload", name=f"wnl{b}")
                for b in range(2)
            ]
            for b in range(2):
                nc.sync.dma_start(wnl[b][:], wn_d[b])
                asum = cpool.tile([C, 1], F32, name=f"asum{b}")
                nc.scalar.activation(wnl[b][:], wnl[b][:], AF.Abs,
                                     accum_out=asum[:])
                sw = cpool.tile([C, 1], F32, name=f"scale_w{b}")
                nc.vector.tensor_scalar_mul(sw[:], asum[:], 1.0 / (9 * C))
                scale_w.append(sw)
                # c1 = pb0 + pb1 (eviction bias), c3 = (1 - alpha) * pb1
                c1 = cpool.tile([C, 1], F32, name=f"c1_{b}")
                nc.vector.tensor_tensor(
                    c1[:], pp_t[:, 5 * b + 2 : 5 * b + 3],
                    pp_t[:, 5 * b + 4 : 5 * b + 5], ALU.add,
                )
                c1s.append(c1)
                apb1 = cpool.tile([C, 1], F32, name=f"apb1_{b}")
                nc.vector.tensor_tensor(
                    apb1[:], pp_t[:, 5 * b + 3 : 5 * b + 4],
                    pp_t[:, 5 * b + 4 : 5 * b + 5], ALU.mult,
                )
                c3 = cpool.tile([C, 1], F32, name=f"c3_{b}")
                nc.vector.tensor_tensor(
                    c3[:], pp_t[:, 5 * b + 4 : 5 * b + 5], apb1[:], ALU.subtract,
                )
                c3s.append(c3)

            # ---------- all remaining pool chunks (both images): keeps
            # image 1's pooling ahead of image 0's epilogues in the engine
            # queues ----------
            sign_todo = {i: [] for i in range(IPC)}
            for i in range(IPC):
                start_k = 2 if i == 0 else 0
                r0 = sum(POOLCHS[:start_k]) if i == 0 else 0
                for k in range(start_k, len(POOLCHS)):
                    eng2 = nc.gpsimd if k % 2 == 1 else nc.vector
                    pool_chunk(i, k, r0, POOLCHS[k], eng2)
                    if i == 0:
                        sign_chunk(i, r0, POOLCHS[k])
                    else:
                        sign_todo[i].append((r0, POOLCHS[k]))
                    r0 += POOLCHS[k]

            for i in range(IPC):
                # ---------- conv: chunk-major, branches interleaved;
                # each half's epilogue fires as soon as its chunks evict ----
                oas = [
                    opool.tile([C, HP, WP], FP16, tag="oasm", name=f"oa{i}{b}")
                    for b in range(2)
                ]
                fos = [
                    outpool.tile([C, HP, WP], F32, tag="fo", name=f"fo{i}{b}")
                    for b in range(2)
                ]

                def epilogue(b, hi):
                    hr0, nr = HALVES[hi]
                    a1 = oas[b][:, hr0 : hr0 + nr, :]
                    a2f = a2pool.tile([C, 35, WP], FP16, tag="a2",
                                      name=f"a2_{i}{b}{hi}")
                    a2 = a2f[:, :nr, :]
                    # a2 = alpha*a1 + (1-alpha)*pb1
                    nc.vector.tensor_scalar(
                        a2, a1, pp_t[:, 5 * b + 3 : 5 * b + 4], c3s[b][:],
                        ALU.mult, ALU.add,
                    )
                    # a1 = max(a1, a2) = prelu(v) + pb1 (fp16 is DVE-only)
                    nc.vector.tensor_tensor(a1, a1, a2, ALU.max)
                    # out = 0.25*pooled + (prelu + pb1)
                    fo = fos[b]
                    nc.vector.scalar_tensor_tensor(
                        fo[:, hr0 : hr0 + nr, :],
                        pooled[i][:, hr0 : hr0 + nr, :], 0.25, a1,
                        ALU.mult, ALU.add,
                    )
                    nc.sync.dma_start(
                        out_d[i, C * b : C * (b + 1), hr0 : hr0 + nr, :],
                        fo[:, hr0 : hr0 + nr, :],
                    )

                prev_half = 0
                for ci, (q0, nrows, ncols, half) in enumerate(CHUNKS):
                    if ci == 6 and i + 1 < IPC:
                        # next image's sign tiles ride the ACT queue here,
                        # between this image's evictions
                        for (sr0, spch) in sign_todo[i + 1]:
                            sign_chunk(i + 1, sr0, spch)
                    if half != prev_half:
                        for b in range(2):
                            epilogue(b, prev_half)
                        prev_half = half
                    last = ci == len(CHUNKS) - 1
                    for b in range(2):
                        pt = pspool.tile(
                            [C, NROWCH * RL], F32, tag="ps",
                            name=f"ps{i}{b}{ci}",
                        )
                        for t in range(9):
                            off = (t // 3) * RL + (t % 3)
                            nc.tensor.matmul(
                                pt[:, :ncols],
                                sgn[b][:, C * t : C * (t + 1)],
                                xbp[i][b][:, q0 + off : q0 + off + ncols],
                                start=(t == 0),
                                stop=(t == 8),
                            )
                        # evict valid columns: a1 = scale_w*S + (pb0 + pb1)
                        cr = q0 // RL
                        nc.scalar.activation(
                            oas[b][:, cr : cr + nrows, :],
                            pt[:, :ncols].rearrange("p (r c) -> p r c", c=RL)[
                                :, :, :WP
                            ],
                            AF.Identity,
                            bias=c1s[b][:],
                            scale=scale_w[b][:],
                        )
                        if last and b == 0:
                            # overlap branch 0's last epilogue with branch
                            # 1's final chunk
                            epilogue(0, 1)
                epilogue(1, 1)

    nc.compile()
    return nc


def _prep_weights(Wb):
    import ml_dtypes

    Wb = np.asarray(Wb, dtype=np.float32)
    wn = Wb.reshape(C, C * 9).astype(ml_dtypes.bfloat16)
    wt = np.ascontiguousarray(
        Wb.reshape(C, C, 9).transpose(1, 2, 0).reshape(C, 9 * C)
    ).astype(ml_dtypes.bfloat16)
    return wt, wn


def _prep_inputs(inputs):
    x = np.ascontiguousarray(np.asarray(inputs["x"], dtype=np.float32))
    wt1, wn1 = _prep_weights(inputs["W1"])
    wt2, wn2 = _prep_weights(inputs["W2"])
    wt = np.ascontiguousarray(np.stack([wt1, wt2]))
    wn = np.ascontiguousarray(np.stack([wn1, wn2]))

    def col(v):
        return np.asarray(v, dtype=np.float32).reshape(C)

    pp = np.zeros((C, 10), dtype=np.float32)
    for b, sfx in enumerate(("1", "2")):
        pp[:, 5 * b + 0] = col(inputs["mvk" + sfx])
        pp[:, 5 * b + 1] = col(inputs["mvb" + sfx])
        pp[:, 5 * b + 2] = col(inputs["pb0_" + sfx])
        pp[:, 5 * b + 3] = col(inputs["alpha" + sfx])
        pp[:, 5 * b + 4] = col(inputs["pb1_" + sfx])
        a = pp[:, 5 * b + 3]
        assert np.all((a >= 0.0) & (a <= 1.0)), (
            "prelu max-identity requires alpha in [0,1]"
        )

    in_maps = [
        {"x": np.ascontiguousarray(x[IPC * c : IPC * (c + 1)]),
         "wt": wt, "wn": wn, "pp": pp}
        for c in range(NCORES)
    ]
    return in_maps


_NC_CACHE = {}


def get_nc():
    if "nc" not in _NC_CACHE:
        _NC_CACHE["nc"] = build_nc()
    return _NC_CACHE["nc"]


def kernel(__trace__=False, **inputs):
    nc = get_nc()
    in_maps = _prep_inputs(inputs)
    res = run_bass_kernel_spmd(
        nc, in_maps, list(range(NCORES)), trace=bool(__trace__)
    )
    out = np.concatenate([res.results[c]["out"] for c in range(NCORES)], axis=0)
    out = np.ascontiguousarray(out.astype(np.float32))
    if __trace__:
        return out, res
    return out


# revision 17
# speedup vs baseline: 1.6104x; 1.0195x over previous
"""Trainium2 Bass kernel for nn_BiSRConv2d_Down.

Reference semantics (forward values):
  out  = avgpool2x2(x)                                  [B, C, H/2, W/2]
  for branch b in {1, 2}:
    xb   = sign(out * mvk_b + mvb_b)                    (tanh STE terms cancel)
    bw   = mean|W_b|_(i,kh,kw) * sign(W_b)              per-output-channel scale
    conv = conv2d(xb, bw, pad=1)
    y_b  = out + (prelu(conv + pb0_b; alpha_b) + pb1_b)
  return concat([y1, y2], channel axis)

Strategy: data-parallel over batch on 8 cores (2 images/core).
 - avg-pool as two tensor-tensor adds (row pairs with contiguous reads, then
   column pairs); the 0.25 scale is folded into the sign activation's scale
   and the final residual op.
 - conv = 9 shifted 128x128 matmuls (channels on partitions), chunk-major
   (9 taps back-to-back into one PSUM bank), branches interleaved per chunk.
   sign activations are exact in bf16 so matmuls accumulate exact integer
   sums; the per-output-channel scale and bias ride the PSUM eviction
   (ScalarE activation with per-partition scale/bias).
 - epilogue: prelu(v)+pb1 = max(v+pb1, alpha*v+pb1) for alpha in [0,1]
   (checked on host). The eviction writes a1 = v+pb1 in fp16, the second arm
   is a2 = alpha*a1 + (1-alpha)*pb1 (fp16 tensor_scalar), then max(a1,a2)
   and one fp32 scalar_tensor_tensor adds the 0.25-scaled pooled residual.
   The conv term is ~2% of the output magnitude so fp16 staging contributes
   ~1e-5 relative error; the residual path stays fp32.
"""

import numpy as np

import concourse.bacc as bacc
import concourse.mybir as mybir
import concourse.tile as tile
from concourse.bass_utils import run_bass_kernel_spmd

F32 = mybir.dt.float32
BF16 = mybir.dt.bfloat16
FP16 = mybir.dt.float16
AF = mybir.ActivationFunctionType
ALU = mybir.AluOpType

B, C, H, W = 16, 128, 128, 128
NCORES = 8
IPC = B // NCORES          # images per core
HP, WP = H // 2, W // 2    # pooled height/width: 64, 64
RL = WP + 2                # padded row stride 66
NPADF = (HP + 2) * RL      # padded image size 4356
XBPLEN = NPADF + 2         # +2 tail pad so tap reads stay in-bounds
NROWCH = 7                 # max output rows per PSUM chunk (7*66=462 <= 512)
# rows per PSUM chunk: small first chunks so the first matmul only needs a
# few pooled rows; grouped into two epilogue halves of 35 + 29 rows
def _mk_chunks(groups):
    out, r0 = [], 0
    for h, grp in enumerate(groups):
        for nr in grp:
            out.append((r0 * RL, nr, nr * RL, h))
            r0 += nr
    return out


# graduated first chunks for image 0 (fast PE start); uniform for the rest
CHUNKS_BY_IMG = [
    _mk_chunks([[2, 5, 7, 7, 7, 7], [7, 7, 7, 7, 1]]),
    _mk_chunks([[7, 7, 7, 7, 7], [7, 7, 7, 7, 1]]),
]
# epilogue halves: output row ranges
HALVES = [(0, 35), (35, 29)]
# pooled-row counts per input chunk: small first chunks let the first sign
# tiles (and therefore the first matmuls) start early
POOLCHS = [2, 2, 4] + [8] * 7


def build_nc():
    nc = bacc.Bacc(
        "TRN2", target_bir_lowering=False, debug=False, num_devices=NCORES
    )
    x_d = nc.dram_tensor("x", [IPC, C, H, W], F32, kind="ExternalInput")
    # wt: host-relaid weights, wt[b][i, t*128+o] = W_b[o, i, kh, kw], t=kh*3+kw
    # (bf16: sign() is exact under bf16 rounding, and mean|W| averages the
    # unbiased rounding error down to ~1e-4 relative, i.e. ~1e-6 on the out)
    wt_d = nc.dram_tensor("wt", [2, C, 9 * C], BF16, kind="ExternalInput")
    # wn: natural weights flattened per output channel (for mean|W|)
    wn_d = nc.dram_tensor("wn", [2, C, 9 * C], BF16, kind="ExternalInput")
    # pp: per-channel params, col 5*b+{0:mvk,1:mvb,2:pb0,3:alpha,4:pb1}
    pp_d = nc.dram_tensor("pp", [C, 10], F32, kind="ExternalInput")
    out_d = nc.dram_tensor("out", [IPC, 2 * C, HP, WP], F32, kind="ExternalOutput")

    with tile.TileContext(nc) as tc:
        with (
            tc.tile_pool(name="const", bufs=1) as cpool,
            tc.tile_pool(name="wload", bufs=2) as wpool,
            tc.tile_pool(name="xin", bufs=3) as xpool,
            tc.tile_pool(name="xsum", bufs=3) as xspool,
            tc.tile_pool(name="oasm", bufs=2) as opool,
            tc.tile_pool(name="a2p", bufs=2) as a2pool,
            tc.tile_pool(name="outp", bufs=2) as outpool,
            tc.tile_pool(name="ps", bufs=8, space="PSUM") as pspool,
        ):
            # ---------- params + branch-0 sign-weights first (they gate the
            # very first matmul; wn/mean|W| prep is deferred) ----------
            pp_t = cpool.tile([C, 10], F32, name="pp_t")
            nc.sync.dma_start(pp_t[:], pp_d[:])

            sgn = [cpool.tile([C, 9 * C], BF16, name=f"sgnw{b}") for b in range(2)]
            wld = [
                wpool.tile([C, 9 * C], BF16, tag="wload", name=f"wld{b}")
                for b in range(2)
            ]
            nc.sync.dma_start(wld[0][:], wt_d[0])
            nc.scalar.sign(sgn[0][:], wld[0][:])

            sc_sign = []
            for b in range(2):
                ss = cpool.tile([C, 1], F32, name=f"sc_sign{b}")
                nc.vector.tensor_scalar_mul(
                    ss[:], pp_t[:, 5 * b + 0 : 5 * b + 1], 0.25
                )
                sc_sign.append(ss)

            # padded sign-activation buffers: only the BORDERS need zeroing
            # (row 0, row 65, cols 0/65 of each row, 2-elem tail); interiors
            # are fully rewritten per image.
            xbp = [
                [cpool.tile([C, XBPLEN], BF16, name=f"xbp{i}{b}") for b in range(2)]
                for i in range(IPC)
            ]
            for i in range(IPC):
                for b in range(2):
                    t = xbp[i][b]
                    nc.vector.memset(t[:, 0:67], 0.0)
                    edge = t[:, 65 : 65 + 65 * RL].rearrange(
                        "p (r c) -> p r c", c=RL
                    )
                    nc.vector.memset(edge[:, :, 0:2], 0.0)
                    nc.vector.memset(t[:, 65 * RL : XBPLEN], 0.0)

            pooled = [
                cpool.tile([C, HP, WP], F32, name=f"pooled{i}") for i in range(IPC)
            ]

            def pool_chunk(i, k, r0, pch, eng2):
                """DMA 2*pch x rows and pool into pooled[i][r0:r0+pch]."""
                rows = slice(r0, r0 + pch)
                xr = xpool.tile([C, 16, W], F32, tag="xr", name=f"xr{i}_{k}")
                nc.sync.dma_start(
                    xr[:, : 2 * pch, :], x_d[i][:, 2 * r0 : 2 * (r0 + pch), :]
                )
                xs = xspool.tile([C, 8, W], F32, tag="xs", name=f"xs{i}_{k}")
                xrr = xr[:, : 2 * pch, :].rearrange(
                    "p (h two) w -> p h two w", two=2
                )
                eng2.tensor_tensor(
                    xs[:, :pch, :], xrr[:, :, 0, :], xrr[:, :, 1, :], ALU.add
                )
                xsw = xs[:, :pch, :].rearrange("p h (w two) -> p h w two", two=2)
                nc.gpsimd.tensor_tensor(
                    pooled[i][:, rows, :], xsw[:, :, :, 0], xsw[:, :, :, 1],
                    ALU.add,
                )

            def sign_chunk(i, r0, pch):
                for b in range(2):
                    xb3 = xbp[i][b][:, :NPADF].rearrange("p (r c) -> p r c", c=RL)
                    nc.scalar.activation(
                        xb3[:, 1 + r0 : 1 + r0 + pch, 1 : 1 + WP],
                        pooled[i][:, r0 : r0 + pch, :],
                        AF.Sign,
                        bias=pp_t[:, 5 * b + 1 : 5 * b + 2],
                        scale=sc_sign[b][:],
                    )

            # first two pool chunks of image 0 interleave with the remaining
            # weight prep, so neither gates the other on DMA/ACT queues
            pool_chunk(0, 0, 0, POOLCHS[0], nc.vector)
            sign_chunk(0, 0, POOLCHS[0])

            nc.sync.dma_start(wld[1][:], wt_d[1])
            nc.scalar.sign(sgn[1][:], wld[1][:])

            pool_chunk(0, 1, POOLCHS[0], POOLCHS[1], nc.vector)
            sign_chunk(0, POOLCHS[0], POOLCHS[1])

            # mean|W| scales + derived per-channel constants
            scale_w, c1s, c3s = [], [], []
            wnl = [
                wpool.tile([C, 9 * C], BF16, tag="wload", name=f"wnl{b}")
                for b in range(2)
            ]
            for b in range(2):
                nc.sync.dma_start(wnl[b][:], wn_d[b])
                asum = cpool.tile([C, 1], F32, name=f"asum{b}")
                nc.scalar.activation(wnl[b][:], wnl[b][:], AF.Abs,
                                     accum_out=asum[:])
                sw = cpool.tile([C, 1], F32, name=f"scale_w{b}")
                nc.vector.tensor_scalar_mul(sw[:], asum[:], 1.0 / (9 * C))
                scale_w.append(sw)
                # c1 = pb0 + pb1 (eviction bias), c3 = (1 - alpha) * pb1
                c1 = cpool.tile([C, 1], F32, name=f"c1_{b}")
                nc.vector.tensor_tensor(
                    c1[:], pp_t[:, 5 * b + 2 : 5 * b + 3],
                    pp_t[:, 5 * b + 4 : 5 * b + 5], ALU.add,
                )
                c1s.append(c1)
                apb1 = cpool.tile([C, 1], F32, name=f"apb1_{b}")
                nc.vector.tensor_tensor(
                    apb1[:], pp_t[:, 5 * b + 3 : 5 * b + 4],
                    pp_t[:, 5 * b + 4 : 5 * b + 5], ALU.mult,
                )
                c3 = cpool.tile([C, 1], F32, name=f"c3_{b}")
                nc.vector.tensor_tensor(
                    c3[:], pp_t[:, 5 * b + 4 : 5 * b + 5], apb1[:], ALU.subtract,
                )
                c3s.append(c3)

            # ---------- all remaining pool chunks (both images): keeps
            # image 1's pooling ahead of image 0's epilogues in the engine
            # queues ----------
            sign_todo = {i: [] for i in range(IPC)}
            for i in range(IPC):
                start_k = 2 if i == 0 else 0
                r0 = sum(POOLCHS[:start_k]) if i == 0 else 0
                for k in range(start_k, len(POOLCHS)):
                    eng2 = nc.gpsimd if k % 2 == 1 else nc.vector
                    pool_chunk(i, k, r0, POOLCHS[k], eng2)
                    if i == 0:
                        sign_chunk(i, r0, POOLCHS[k])
                    else:
                        sign_todo[i].append((r0, POOLCHS[k]))
                    r0 += POOLCHS[k]

            for i in range(IPC):
                # ---------- conv: chunk-major, branches interleaved;
                # each half's epilogue fires as soon as its chunks evict ----
                oas = [
                    opool.tile([C, HP, WP], FP16, tag="oasm", name=f"oa{i}{b}")
                    for b in range(2)
                ]
                fos = [
                    outpool.tile([C, HP, WP], F32, tag="fo", name=f"fo{i}{b}")
                    for b in range(2)
                ]

                def epilogue(b, hi, split=1):
                    hr0, nr = HALVES[hi]
                    a1 = oas[b][:, hr0 : hr0 + nr, :]
                    a2f = a2pool.tile([C, 35, WP], FP16, tag="a2",
                                      name=f"a2_{i}{b}{hi}")
                    a2 = a2f[:, :nr, :]
                    # a2 = alpha*a1 + (1-alpha)*pb1
                    nc.vector.tensor_scalar(
                        a2, a1, pp_t[:, 5 * b + 3 : 5 * b + 4], c3s[b][:],
                        ALU.mult, ALU.add,
                    )
                    # a1 = max(a1, a2) = prelu(v) + pb1 (fp16 is DVE-only)
                    nc.vector.tensor_tensor(a1, a1, a2, ALU.max)
                    # out = 0.25*pooled + (prelu + pb1); split so the final
                    # store overlaps the compute on the critical tail
                    fo = fos[b]
                    bounds = [hr0 + nr * s // split for s in range(split + 1)]
                    for s in range(split):
                        p0, p1 = bounds[s], bounds[s + 1]
                        nc.vector.scalar_tensor_tensor(
                            fo[:, p0:p1, :], pooled[i][:, p0:p1, :], 0.25,
                            oas[b][:, p0:p1, :], ALU.mult, ALU.add,
                        )
                        nc.sync.dma_start(
                            out_d[i, C * b : C * (b + 1), p0:p1, :],
                            fo[:, p0:p1, :],
                        )

                chunks_i = CHUNKS_BY_IMG[i]

                def conv_chunk(ci, b):
                    q0, nrows, ncols, _half = chunks_i[ci]
                    pt = pspool.tile(
                        [C, NROWCH * RL], F32, tag="ps", name=f"ps{i}{b}{ci}"
                    )
                    for t in range(9):
                        off = (t // 3) * RL + (t % 3)
                        nc.tensor.matmul(
                            pt[:, :ncols],
                            sgn[b][:, C * t : C * (t + 1)],
                            xbp[i][b][:, q0 + off : q0 + off + ncols],
                            start=(t == 0),
                            stop=(t == 8),
                        )
                    # evict valid columns: a1 = scale_w*S + (pb0 + pb1)
                    cr = q0 // RL
                    nc.scalar.activation(
                        oas[b][:, cr : cr + nrows, :],
                        pt[:, :ncols].rearrange("p (r c) -> p r c", c=RL)[
                            :, :, :WP
                        ],
                        AF.Identity,
                        bias=c1s[b][:],
                        scale=scale_w[b][:],
                    )

                last_img = i == IPC - 1
                h1_start = next(
                    ci for ci, c in enumerate(chunks_i) if c[3] == 1
                )
                for ci in range(h1_start):
                    if ci == 4 and i + 1 < IPC:
                        # next image's sign tiles ride the ACT queue here,
                        # between this image's evictions
                        for (sr0, spch) in sign_todo[i + 1]:
                            sign_chunk(i + 1, sr0, spch)
                    for b in range(2):
                        conv_chunk(ci, b)
                for b in range(2):
                    epilogue(b, 0)
                if not last_img:
                    for ci in range(h1_start, len(chunks_i)):
                        for b in range(2):
                            conv_chunk(ci, b)
                    for b in range(2):
                        epilogue(b, 1)
                else:
                    # de-interleave the final half: branch 0 finishes early
                    # so its epilogue overlaps branch 1's matmuls
                    for ci in range(h1_start, len(chunks_i)):
                        conv_chunk(ci, 0)
                    epilogue(0, 1)
                    for ci in range(h1_start, len(chunks_i)):
                        conv_chunk(ci, 1)
                    epilogue(1, 1, split=2)

    nc.compile()
    return nc


def _prep_weights(Wb):
    import ml_dtypes

    Wb = np.asarray(Wb, dtype=np.float32)
    wn = Wb.reshape(C, C * 9).astype(ml_dtypes.bfloat16)
    wt = np.ascontiguousarray(
        Wb.reshape(C, C, 9).transpose(1, 2, 0).reshape(C, 9 * C)
    ).astype(ml_dtypes.bfloat16)
    return wt, wn


def _prep_inputs(inputs):
    x = np.ascontiguousarray(np.asarray(inputs["x"], dtype=np.float32))
    wt1, wn1 = _prep_weights(inputs["W1"])
    wt2, wn2 = _prep_weights(inputs["W2"])
    wt = np.ascontiguousarray(np.stack([wt1, wt2]))
    wn = np.ascontiguousarray(np.stack([wn1, wn2]))

    def col(v):
        return np.asarray(v, dtype=np.float32).reshape(C)

    pp = np.zeros((C, 10), dtype=np.float32)
    for b, sfx in enumerate(("1", "2")):
        pp[:, 5 * b + 0] = col(inputs["mvk" + sfx])
        pp[:, 5 * b + 1] = col(inputs["mvb" + sfx])
        pp[:, 5 * b + 2] = col(inputs["pb0_" + sfx])
        pp[:, 5 * b + 3] = col(inputs["alpha" + sfx])
        pp[:, 5 * b + 4] = col(inputs["pb1_" + sfx])
        a = pp[:, 5 * b + 3]
        assert np.all((a >= 0.0) & (a <= 1.0)), (
            "prelu max-identity requires alpha in [0,1]"
        )

    in_maps = [
        {"x": np.ascontiguousarray(x[IPC * c : IPC * (c + 1)]),
         "wt": wt, "wn": wn, "pp": pp}
        for c in range(NCORES)
    ]
    return in_maps


_NC_CACHE = {}


def get_nc():
    if "nc" not in _NC_CACHE:
        _NC_CACHE["nc"] = build_nc()
    return _NC_CACHE["nc"]


def kernel(__trace__=False, **inputs):
    nc = get_nc()
    in_maps = _prep_inputs(inputs)
    res = run_bass_kernel_spmd(
        nc, in_maps, list(range(NCORES)), trace=bool(__trace__)
    )
    out = np.concatenate([res.results[c]["out"] for c in range(NCORES)], axis=0)
    out = np.ascontiguousarray(out.astype(np.float32))
    if __trace__:
        return out, res
    return out
